# revision 49
# baseline (speedup 1.0000x reference)
"""Ragged sequence assembly on 8 TRN2 NeuronCores.

out[b] = concat([CLS, X[b, :lx[b]], RING, Xr[b, :lr[b]], END]) padded
with zeros to T = LX + LR + 3 rows of D floats.

Strategy: data-parallel over B (2 samples/core). Pure DRAM->DRAM DMA.

The host prepends CLS to each X sample and RING to each Xr sample, so
the output is two contiguous ragged segments plus END:
  seg1 = [CLS, X[b,:lx]]   -> out rows [0, 1+lx)      (src == dst offset)
  seg2 = [RING, Xr[b,:lr]] -> out rows [1+lx, 2+lx+lr) (dst = src+(1+lx))
  END                      -> out row 2+lx+lr
Each ragged segment is a branch-free binary decomposition of its length
len: one DMA per bit, LSB-first tiling (block k sits at len mod 2^k =
len & (2^k - 1), so every DMA's offsets are computed independently with
5-6 ALU ops straight into that DMA's private registers). Bit 0 runs
unconditionally (its block is always at offset 0 with content CLS/RING
- a benign identical overlap); the top bit only fires when len is
exactly 2^top, i.e. offset 0 with a static source. Issue order is
DESCENDING so multi-MB blocks enter the DMA queues first. A cleared bit
adds 2^30 to the dst offset, failing the runtime bounds check: with
bounds_check="skip_entire_dma" the DMA is skipped but its semaphore
still increments, so completion counts stay static; skipped sources
stay in bounds because len mod 2^k + 2^k <= 2^(k+1)-1.

Engine split (the usable sequencer register pool is 49, HWDGE/SWDGE
read offset registers asynchronously after issue - so offsets live in
private never-rewritten registers - and a fired HWDGE dma_start costs
the issuing sequencer ~740ns, making issue bandwidth a real resource):
SP carries seg1 of sample 0 plus seg2 big bits (10, 9..7) and END of
sample 1; ACT the converse ("crossed", so one queue never carries a
whole big sample). Pool/SWDGE gets the six tiny seg2 bits (6..1) of
both samples - a third issue queue whose skipped DMAs cost ~5ns.
Tensors are flat int8 byte views (no stride-multiply lowering temps)
with per-sample parameters (no sample-base temps). With issue spread
this way the kernel is drain-bound: the 16 SDMA engines sustain ~460
GB/s of combined read+write for DRAM->DRAM copies.

The zero padding is never written: run_bass_kernel_spmd pre-zeros
ExternalOutput buffers (the PJRT path donates zeroed buffers —
bass2jax.py documents kernels rely on this).
"""

import sys

if "/opt/trn_rl_repo" not in sys.path:
    sys.path.insert(0, "/opt/trn_rl_repo")

import numpy as np

import concourse.bass as bass
import concourse.mybir as mybir
from concourse.bass_utils import run_bass_kernel_spmd

B, LX, LR, D = 16, 2048, 1024, 768
T = LX + LR + 3
RB = D * 4  # bytes per row
OOB_HUGE = 1 << 30  # far beyond any tensor extent; marks a skipped DMA
N_CORES = 8
PER_CORE = B // N_CORES  # 2

I8 = mybir.dt.int8
I32 = mybir.dt.int32

SEG2_SPLIT = 7  # seg2 bits >= SEG2_SPLIT on SP/ACT, bits 6..1 on Pool


def _emit_seg2_tail(eng, s, lens_sb, Xrp, out, sem):
    """seg2 tiny bits (SEG2_SPLIT-1 .. 1) for local sample s on Pool,
    same LSB-first tiling (block k at len2 mod 2^k)."""
    n = 0
    l2_r = eng.alloc_register(f"pl2_{s}")
    do_r = eng.alloc_register(f"pdo_{s}")
    eng.reg_load([do_r, l2_r], lens_sb[0:1, 2 * s : 2 * s + 2])
    eng.reg_alu(do_r, do_r, RB, mybir.AluOpType.mult)  # (1+lx)*RB, read-only
    for k in range(SEG2_SPLIT - 1, 0, -1):
        nbytes = (1 << k) * RB
        anns = ((1 << k) - 1) * RB
        annd = (1 << 11) * RB + anns
        p_r = eng.alloc_register(f"pp_{s}_{k}")
        q_r = eng.alloc_register(f"pq_{s}_{k}")
        eng.reg_alu(p_r, l2_r, (1 << k) - 1, mybir.AluOpType.bitwise_and)
        eng.reg_alu(p_r, p_r, RB, mybir.AluOpType.mult)
        eng.reg_alu(q_r, l2_r, 1 << k, mybir.AluOpType.bitwise_and)
        eng.reg_alu(q_r, q_r, 0, mybir.AluOpType.is_equal)
        eng.reg_alu(q_r, q_r, OOB_HUGE, mybir.AluOpType.mult)
        eng.reg_alu(q_r, q_r, p_r, mybir.AluOpType.add)
        eng.reg_alu(q_r, q_r, do_r, mybir.AluOpType.add)
        q = eng.snap(q_r, donate=True, min_val=0, max_val=annd)
        p = eng.snap(p_r, donate=True, min_val=0, max_val=anns)
        eng.dma_start(
            out[0][bass.ds(q, nbytes)],
            Xrp[0][bass.ds(p, nbytes)],
            bounds_check="skip_entire_dma",
        ).then_inc(sem, 16)
        n += 1
    return n


def _emit_main(eng, sa, sb, lens_sb, Xp, Xrp, END, out_a, out_b, sem):
    """seg1 of local sample sa plus seg2 big bits + END of local sample
    sb on one HWDGE engine. Crossing sa != sb keeps each queue's load at
    one seg1 + one seg2 even when a core's samples are very uneven.

    Offsets use the LSB-first tiling: block k sits at len mod 2^k, so
    each DMA's offsets are computed independently (5 ALU ops) straight
    into that DMA's private registers - no accumulation chain, no snap
    copies. Issue order is descending so big blocks enter the queue
    first. Returns DMA count."""
    n = 0
    l1_r = eng.alloc_register(f"l1_{sa}")
    l2_r = eng.alloc_register(f"l2_{sa}")
    do_r = eng.alloc_register(f"do_{sa}")
    eng.reg_load(l1_r, lens_sb[0:1, 2 * sa : 2 * sa + 1])

    # ---- seg1: len1 = 1+lx in [1, 2048], bits 0..11 ----
    # bit 0 unconditionally: row 0 <- Xp[0] == CLS is always correct
    eng.dma_start(out_a[0][0:RB], Xp[0][0:RB]).then_inc(sem, 16)
    n += 1
    # bit 11 fires only when len1 == 2048 -> offset 0, static src
    q_r = eng.alloc_register(f"q11_{sa}")
    eng.reg_alu(q_r, l1_r, 1 << 11, mybir.AluOpType.bitwise_and)
    eng.reg_alu(q_r, q_r, 0, mybir.AluOpType.is_equal)
    eng.reg_alu(q_r, q_r, OOB_HUGE, mybir.AluOpType.mult)
    q = eng.snap(q_r, donate=True, min_val=0, max_val=RB)
    eng.dma_start(
        out_a[0][bass.ds(q, (1 << 11) * RB)],
        Xp[0][0 : (1 << 11) * RB],
        bounds_check="skip_entire_dma",
    ).then_inc(sem, 16)
    n += 1
    # bits 10..1 descending; src == dst offset == (len1 mod 2^k)*RB
    for k in range(10, 0, -1):
        nbytes = (1 << k) * RB
        ann = ((1 << k) - 1) * RB
        p_r = eng.alloc_register(f"p1_{sa}_{k}")
        q_r = eng.alloc_register(f"q1_{sa}_{k}")
        eng.reg_alu(p_r, l1_r, ((1 << k) - 1) * RB // RB, mybir.AluOpType.bitwise_and)
        eng.reg_alu(p_r, p_r, RB, mybir.AluOpType.mult)
        eng.reg_alu(q_r, l1_r, 1 << k, mybir.AluOpType.bitwise_and)
        eng.reg_alu(q_r, q_r, 0, mybir.AluOpType.is_equal)
        eng.reg_alu(q_r, q_r, OOB_HUGE, mybir.AluOpType.mult)
        eng.reg_alu(q_r, q_r, p_r, mybir.AluOpType.add)
        q = eng.snap(q_r, donate=True, min_val=0, max_val=ann)
        p = eng.snap(p_r, donate=True, min_val=0, max_val=ann)
        eng.dma_start(
            out_a[0][bass.ds(q, nbytes)],
            Xp[0][bass.ds(p, nbytes)],
            bounds_check="skip_entire_dma",
        ).then_inc(sem, 16)
        n += 1

    # ---- seg2 big bits of sample sb: len2 in [1, 1024], bits 0..10 ----
    eng.reg_load([l1_r, l2_r], lens_sb[0:1, 2 * sb : 2 * sb + 2])
    eng.reg_alu(do_r, l1_r, RB, mybir.AluOpType.mult)  # (1+lx)*RB, read-only
    # bit 0 unconditionally: row 1+lx <- Xrp[0] == RING always correct
    doff = eng.snap(do_r, donate=False, min_val=0, max_val=(1 + LX) * RB)
    eng.dma_start(out_b[0][bass.ds(doff, RB)], Xrp[0][0:RB]).then_inc(sem, 16)
    n += 1
    # bit 10 fires only when len2 == 1024 -> offset 0, static src
    q_r = eng.alloc_register(f"q10_{sa}")
    eng.reg_alu(q_r, l2_r, 1 << 10, mybir.AluOpType.bitwise_and)
    eng.reg_alu(q_r, q_r, 0, mybir.AluOpType.is_equal)
    eng.reg_alu(q_r, q_r, OOB_HUGE, mybir.AluOpType.mult)
    eng.reg_alu(q_r, q_r, do_r, mybir.AluOpType.add)
    q = eng.snap(q_r, donate=True, min_val=0, max_val=(1 + LX) * RB)
    eng.dma_start(
        out_b[0][bass.ds(q, (1 << 10) * RB)],
        Xrp[0][0 : (1 << 10) * RB],
        bounds_check="skip_entire_dma",
    ).then_inc(sem, 16)
    n += 1
    # bits 9..SEG2_SPLIT descending; dst = doff + (len2 mod 2^k)*RB
    for k in range(9, SEG2_SPLIT - 1, -1):
        nbytes = (1 << k) * RB
        anns = ((1 << k) - 1) * RB
        annd = (1 << 11) * RB + anns
        p_r = eng.alloc_register(f"p2_{sa}_{k}")
        q_r = eng.alloc_register(f"q2_{sa}_{k}")
        eng.reg_alu(p_r, l2_r, (1 << k) - 1, mybir.AluOpType.bitwise_and)
        eng.reg_alu(p_r, p_r, RB, mybir.AluOpType.mult)
        eng.reg_alu(q_r, l2_r, 1 << k, mybir.AluOpType.bitwise_and)
        eng.reg_alu(q_r, q_r, 0, mybir.AluOpType.is_equal)
        eng.reg_alu(q_r, q_r, OOB_HUGE, mybir.AluOpType.mult)
        eng.reg_alu(q_r, q_r, p_r, mybir.AluOpType.add)
        eng.reg_alu(q_r, q_r, do_r, mybir.AluOpType.add)
        q = eng.snap(q_r, donate=True, min_val=0, max_val=annd)
        p = eng.snap(p_r, donate=True, min_val=0, max_val=anns)
        eng.dma_start(
            out_b[0][bass.ds(q, nbytes)],
            Xrp[0][bass.ds(p, nbytes)],
            bounds_check="skip_entire_dma",
        ).then_inc(sem, 16)
        n += 1

    # ---- END -> row 2+lx+lr == (len1 + len2) rows in ----
    d_r = eng.alloc_register(f"de_{sa}")
    eng.reg_alu(d_r, l1_r, l2_r, mybir.AluOpType.add)
    eng.reg_alu(d_r, d_r, RB, mybir.AluOpType.mult)
    doff = eng.snap(d_r, donate=True, min_val=0, max_val=(T - 1) * RB)
    eng.dma_start(out_b[0][bass.ds(doff, RB)], END[0][0:RB]).then_inc(sem, 16)
    n += 1
    return n


def build_program() -> bass.Bass:
    # Note on the ~7.4us head before the first DMA: it is NEFF/engine
    # BOOT latency, not the Bass-constructor all-engine barrier - a
    # build whose SP/ACT skip that barrier still issues its first DMA at
    # t=7.4us (measured). The barrier merely absorbs boot stagger, so it
    # stays (stock, battle-tested path).
    nc = bass.Bass()

    X0 = nc.declare_dram_parameter("X0", [1, (1 + LX) * RB], I8, isOutput=False)
    X1 = nc.declare_dram_parameter("X1", [1, (1 + LX) * RB], I8, isOutput=False)
    Xr0 = nc.declare_dram_parameter("Xr0", [1, (1 + LR) * RB], I8, isOutput=False)
    Xr1 = nc.declare_dram_parameter("Xr1", [1, (1 + LR) * RB], I8, isOutput=False)
    END = nc.declare_dram_parameter("END", [1, RB], I8, isOutput=False)
    lens = nc.declare_dram_parameter("lens", [1, 2 * PER_CORE], I32, isOutput=False)
    out0 = nc.declare_dram_parameter("out0", [1, T * RB], I8, isOutput=True)
    out1 = nc.declare_dram_parameter("out1", [1, T * RB], I8, isOutput=True)

    with (
        nc.sbuf_tensor([1, 2 * PER_CORE], I32) as lens_sb,
        nc.semaphore("lens_sem") as lens_sem,
        nc.semaphore("sp_sem") as sp_sem,
        nc.semaphore("act_sem") as act_sem,
        nc.semaphore("pool_sem") as pool_sem,
        nc.Block() as block,
    ):

        @block.sync
        def _(sync):
            sync.dma_start(lens_sb[:, :], lens[:, :]).then_inc(lens_sem, 16)
            sync.wait_ge(lens_sem, 16)
            n = _emit_main(sync, 0, 1, lens_sb, X0, Xr1, END, out0, out1, sp_sem)
            sync.wait_ge(sp_sem, n * 16)

        @block.scalar
        def _(scalar):
            scalar.wait_ge(lens_sem, 16)
            n = _emit_main(scalar, 1, 0, lens_sb, X1, Xr0, END, out1, out0, act_sem)
            scalar.wait_ge(act_sem, n * 16)

        @block.gpsimd
        def _(gpsimd):
            gpsimd.wait_ge(lens_sem, 16)
            n = _emit_seg2_tail(gpsimd, 0, lens_sb, Xr0, out0, pool_sem)
            n += _emit_seg2_tail(gpsimd, 1, lens_sb, Xr1, out1, pool_sem)
            gpsimd.wait_ge(pool_sem, n * 16)

    return nc


_NC_CACHE: list = []


def _get_nc() -> bass.Bass:
    if not _NC_CACHE:
        _NC_CACHE.append(build_program())
    return _NC_CACHE[0]


def _balance_order(lx: np.ndarray, lr: np.ndarray) -> np.ndarray:
    """Pair samples to minimize the max per-core total copy length:
    greedy largest-with-smallest, then local-search swaps."""
    tot = (lx.astype(np.int64) + lr.astype(np.int64)).ravel()
    srt = np.argsort(tot)
    pairs = [[int(srt[i]), int(srt[B - 1 - i])] for i in range(B // 2)]

    def cost():
        return sum((tot[a] + tot[b]) ** 2 for a, b in pairs)

    improved = True
    while improved:
        improved = False
        for i in range(len(pairs)):
            for j in range(i + 1, len(pairs)):
                for ii in range(2):
                    for jj in range(2):
                        old = cost()
                        pairs[i][ii], pairs[j][jj] = pairs[j][jj], pairs[i][ii]
                        if cost() < old:
                            improved = True
                        else:
                            pairs[i][ii], pairs[j][jj] = (
                                pairs[j][jj],
                                pairs[i][ii],
                            )
    order = np.empty(B, dtype=np.int64)
    for i, (a, b) in enumerate(pairs):
        order[2 * i] = a
        order[2 * i + 1] = b
    return order


def kernel(X, Xr, CLS, RING, END, lx, lr, _trace=False, _trace_kwargs=None):
    X = np.ascontiguousarray(X, dtype=np.float32)
    Xr = np.ascontiguousarray(Xr, dtype=np.float32)
    CLS = np.ascontiguousarray(CLS, dtype=np.float32).reshape(1, D)
    RING = np.ascontiguousarray(RING, dtype=np.float32).reshape(1, D)
    END = np.ascontiguousarray(END, dtype=np.float32).reshape(1, D)
    lx = np.asarray(lx, dtype=np.int32)
    lr = np.asarray(lr, dtype=np.int32)

    # [CLS; X[b]] and [RING; Xr[b]] as flat byte rows, per sample
    Xp = np.concatenate(
        [np.broadcast_to(CLS[None], (B, 1, D)), X], axis=1
    ).reshape(B, -1).view(np.int8)
    Xrp = np.concatenate(
        [np.broadcast_to(RING[None], (B, 1, D)), Xr], axis=1
    ).reshape(B, -1).view(np.int8)
    ENDb = END.reshape(1, -1).view(np.int8)

    order = _balance_order(lx, lr)

    in_maps = []
    for c in range(N_CORES):
        ids = order[c * PER_CORE : (c + 1) * PER_CORE]
        lens = np.empty((1, 2 * PER_CORE), dtype=np.int32)
        for i, b in enumerate(ids):
            lens[0, 2 * i] = 1 + lx[b]
            lens[0, 2 * i + 1] = 1 + lr[b]
        in_maps.append(
            {
                "X0": Xp[ids[0] : ids[0] + 1],
                "X1": Xp[ids[1] : ids[1] + 1],
                "Xr0": Xrp[ids[0] : ids[0] + 1],
                "Xr1": Xrp[ids[1] : ids[1] + 1],
                "END": ENDb,
                "lens": lens,
            }
        )

    nc = _get_nc()
    kres = run_bass_kernel_spmd(
        nc,
        in_maps,
        core_ids=list(range(N_CORES)),
        trace=_trace,
        **(_trace_kwargs or {}),
    )

    out = np.empty((B, T, D), dtype=np.float32)
    for c in range(N_CORES):
        ids = order[c * PER_CORE : (c + 1) * PER_CORE]
        for i, b in enumerate(ids):
            res = np.ascontiguousarray(kres.results[c][f"out{i}"]).view(np.float32)
            out[b] = res.reshape(T, D)

    if _trace:
        return out, kres
    return out
